# revision 21
# baseline (speedup 1.0000x reference)
"""GQA self-attention (B=2, S=2048, H=2048, NQ=16, NKV=4, D=128) on 8 TRN2
NeuronCores.

Sharding: core = (batch, kv-group): 2 batches x 4 kv heads. Each core computes
its batch's 4 q heads + 1 kv head end-to-end (q/k/v proj + rope + causal
attention + partial o_proj against the matching 512-column slice of Wo).
Host sums the 4 partial o_proj outputs per batch (the "all-reduce") and
reassembles new_k / new_v.

Device-side layout: everything feature-on-partitions. Host pre-transposes
x -> xT [H, S] and weights -> W^T so every matmul contraction dim is the
partition dim. Matmul operands are bf16 (fp32 accumulation in PSUM): enables
fast-weight-load and halves DMA; new_k/new_v are produced from the fp32
pre-cast intermediates so the cache outputs keep full precision. Attention
scores are computed transposed ([sk, sq]) so the exp'd probabilities feed the
A@V matmul directly as the moving operand; the softmax denominator comes from
a ones-stationary matmul accumulated in PSUM (broadcast across partitions for
a full-width reciprocal+scale). Softmax skips the max-subtraction
(scores*scale are O(10); exp cannot overflow and softmax is shift-invariant).
V is projected transposed like q/k and flipped back to [s, d] via PE
transposes (full-width N=512 matmuls instead of N=128 ones).
"""

import os
import sys

for _p in ("/opt/trn_rl_repo",):
    if _p not in sys.path and os.path.isdir(_p):
        sys.path.insert(0, _p)

import numpy as np
import ml_dtypes

B, S, H = 2, 2048, 2048
NQ, NKV, D = 16, 4, 128
N_CORES = 8
HD = 4          # q heads per core
KC = H // 128   # 16 hidden chunks
SC = S // 512   # 4 sequence 512-chunks
SB = S // 128   # 16 sequence 128-blocks
SCALE = 1.0 / np.sqrt(D).astype(np.float32)
NEG = -1.0e30

_CACHE = {}
LAST_EXEC_NS = None


def _build_nc():
    from contextlib import ExitStack

    from concourse import bacc, tile
    import concourse.mybir as mybir

    f32 = mybir.dt.float32
    bf16 = mybir.dt.bfloat16
    AF = mybir.ActivationFunctionType

    nc = bacc.Bacc("TRN2", target_bir_lowering=False, debug=False,
                   num_devices=N_CORES)

    def din(name, shape, dt=f32):
        return nc.dram_tensor(name, list(shape), dt, kind="ExternalInput").ap()

    def dout(name, shape, dt=f32):
        return nc.dram_tensor(name, list(shape), dt, kind="ExternalOutput").ap()

    xT = din("xT", (H, S), bf16)          # x[b].T
    # weights pre-tiled on host so each DMA moves >=1KB-contiguous lines
    wqt = din("wqt", (8, D, 2, HD * D), bf16)   # pairs of h-chunks of Wq.T
    wkt = din("wkt", (4, D, 4, D), bf16)        # quads of h-chunks of Wk.T
    wvt = din("wvt", (4, D, 4, D), bf16)
    wot = din("wot", (4, D, 4, 512), bf16)      # [dq-chunk][d][ec][e']
    bq = din("bq", (D, HD))               # bq[g-slice] as [d, head]
    bk = din("bk", (D, 1))
    bv = din("bv", (D, 1))
    cosT = din("cosT", (D, S))            # rope_cos[b].T
    sinT = din("sinT", (D, S))            # rope_sin[b].T, rows 0:64 negated
    maskT = din("maskT", (4, D, 512))     # causal mask tiles for diag blocks
    ones = din("ones", (D, D), bf16)
    ident = din("ident", (D, D))          # identity for PE transposes

    outp = dout("outp", (S, H))           # partial out[b] (this group's slice)
    kT_out = dout("kT_out", (D, S))       # rope'd k, transposed, fp32
    vT_out = dout("vT_out", (D, S))       # v, transposed, fp32

    with tile.TileContext(nc) as tc, ExitStack() as ctx:
        pool = ctx.enter_context(tc.tile_pool(name="persist", bufs=1))

        wq_s = []
        for kp in range(8):
            w = pool.tile([128, 2, HD * D], bf16, tag=f"wq{kp}", name=f"wq{kp}")
            nc.gpsimd.dma_start(w[:], wqt[kp])
            wq_s.append(w)
        wk_s = []
        wv_s = []
        for kp in range(4):
            w = pool.tile([128, 4, D], bf16, tag=f"wk{kp}", name=f"wk{kp}")
            nc.scalar.dma_start(w[:], wkt[kp])
            wk_s.append(w)
            w = pool.tile([128, 4, D], bf16, tag=f"wv{kp}", name=f"wv{kp}")
            nc.scalar.dma_start(w[:], wvt[kp])
            wv_s.append(w)

        def wq_sl(kc, h):
            return wq_s[kc // 2][:, kc % 2, 128 * h:128 * (h + 1)]

        def wk_sl(kc):
            return wk_s[kc // 4][:, kc % 4, :]

        def wv_sl(kc):
            return wv_s[kc // 4][:, kc % 4, :]
        cos_s = pool.tile([128, S], f32)
        sin_s = pool.tile([128, S], f32)
        mask_s = pool.tile([128, 4, 512], f32)
        ones_s = pool.tile([128, D], bf16)
        ident_s = pool.tile([128, D], f32)
        bq_s = pool.tile([128, HD], f32)
        bk_s = pool.tile([128, 1], f32)
        bv_s = pool.tile([128, 1], f32)

        def load_consts():
            # issued from the scalar sequencer so they do
            # not delay the SP-issued xt loads that gate the first matmuls
            nc.scalar.dma_start(bq_s[:], bq[:, :])
            nc.scalar.dma_start(bk_s[:], bk[:, :])
            nc.scalar.dma_start(bv_s[:], bv[:, :])
            nc.scalar.dma_start(cos_s[:], cosT[:, :])
            nc.scalar.dma_start(sin_s[:], sinT[:, :])
            nc.scalar.dma_start(ident_s[:], ident[:, :])
            nc.scalar.dma_start(mask_s[:], maskT.rearrange("j p f -> p j f"))
            nc.scalar.dma_start(ones_s[:], ones[:, :])

        # chunked activations (per-512-chunk tiles -> fine-grained deps)
        qt = [[pool.tile([128, 512], bf16, tag=f"qt{h}_{c}", name=f"qt{h}_{c}")
               for c in range(SC)] for h in range(HD)]
        ktf = [pool.tile([128, 512], f32, tag=f"ktf{c}", name=f"ktf{c}")
               for c in range(SC)]
        kt16 = [pool.tile([128, 512], bf16, tag=f"kt16_{c}", name=f"kt16_{c}")
                for c in range(SC)]
        vtf = [pool.tile([128, 512], f32, tag=f"vtf{c}", name=f"vtf{c}")
               for c in range(SC)]
        v_s = [pool.tile([128, D], bf16, tag=f"v{sb}", name=f"v{sb}")
               for sb in range(SB)]
        ot = [[pool.tile([128, 512], bf16, tag=f"ot{h}_{c}", name=f"ot{h}_{c}")
               for c in range(SC)] for h in range(HD)]

        tmp_pool = ctx.enter_context(tc.tile_pool(name="tmp", bufs=2))

        def rope_evac(dst, psum, bias_ap, cs, nm):
            # dst = (psum + bias) * cos + swap_halves(psum + bias) * sin_signed
            qb = tmp_pool.tile([128, 512], f32, tag="ropeqb", name=f"qb_{nm}")
            nc.scalar.activation(qb[:], psum, AF.Identity, bias=bias_ap)
            qsw = tmp_pool.tile([128, 512], f32, tag="ropesw", name=f"sw_{nm}")
            nc.gpsimd.dma_start(qsw[0:64, :], qb[64:128, :])
            nc.gpsimd.dma_start(qsw[64:128, :], qb[0:64, :])
            t = tmp_pool.tile([128, 512], f32, tag="ropet", name=f"t_{nm}")
            nc.vector.tensor_mul(t[:], qsw[:], sin_s[:, cs])
            t2 = tmp_pool.tile([128, 512], f32, tag="ropet2", name=f"t2_{nm}")
            nc.vector.tensor_mul(t2[:], qb[:], cos_s[:, cs])
            nc.vector.tensor_add(dst, t2[:], t[:])

        # ---------------- Phase A: q/k/v projections + rope ----------------
        phaseA = ExitStack()
        xt_pool = phaseA.enter_context(tc.tile_pool(name="xt", bufs=32))
        psA = phaseA.enter_context(tc.tile_pool(name="psA", bufs=1, space="PSUM"))
        psT = phaseA.enter_context(tc.tile_pool(name="psT", bufs=2, space="PSUM"))
        # issue ALL xt loads upfront: ring FIFO keeps cpair0 first, and
        # cpair1 prefetches ~40us before its matmuls need it
        xts_all = []
        for cpair in range(SC // 2):
            xts = []
            for kc in range(KC):
                xt = xt_pool.tile([128, 1024], bf16, tag="xt",
                                  name=f"xt{kc}_{cpair}")
                nc.sync.dma_start(
                    xt[:], xT[128 * kc:128 * (kc + 1),
                              1024 * cpair:1024 * (cpair + 1)])
                xts.append(xt)
            xts_all.append(xts)
            if cpair == 0:
                load_consts()
        for cpair in range(SC // 2):
          xts = xts_all[cpair]
          for ci in range(2):
            c = 2 * cpair + ci
            cs = slice(512 * c, 512 * (c + 1))
            pq = [psA.tile([128, 512], f32, tag=f"pq{h}", name=f"pq{h}_{c}")
                  for h in range(HD)]
            pk = psA.tile([128, 512], f32, tag="pk", name=f"pk_{c}")
            pv = psA.tile([128, 512], f32, tag="pv", name=f"pv_{c}")
            for kc in range(KC):
                st, sp = kc == 0, kc == KC - 1
                xr = xts[kc][:, 512 * ci:512 * (ci + 1)]
                for h in range(HD):
                    nc.tensor.matmul(pq[h][:], wq_sl(kc, h), xr,
                                     start=st, stop=sp)
                nc.tensor.matmul(pk[:], wk_sl(kc), xr, start=st, stop=sp)
                nc.tensor.matmul(pv[:], wv_sl(kc), xr, start=st, stop=sp)
            for h in range(HD):
                rope_evac(qt[h][c][:], pq[h][:], bq_s[:, h:h + 1], cs,
                          f"q{h}_{c}")
            rope_evac(ktf[c][:], pk[:], bk_s[:, :], cs, f"k_{c}")
            nc.vector.tensor_copy(kt16[c][:], ktf[c][:])
            nc.gpsimd.dma_start(kT_out[:, cs], ktf[c][:])
            # v: add bias on evac, write fp32 cache copy, then PE-transpose
            # each 128-block into [s, d] layout for the A@V stationary.
            nc.scalar.activation(vtf[c][:], pv[:], AF.Identity, bias=bv_s[:, :])
            nc.gpsimd.dma_start(vT_out[:, cs], vtf[c][:])
            for j in range(4):
                ptr = psT.tile([128, D], f32, tag="ptr", name=f"ptr{c}_{j}")
                nc.tensor.transpose(ptr[:], vtf[c][:, 128 * j:128 * (j + 1)],
                                    ident_s[:])
                nc.scalar.copy(v_s[4 * c + j][:].bitcast(bf16), ptr[:])
        phaseA.close()

        # ---------------- Phase B: causal attention per head ----------------
        phaseB = ExitStack()
        # prefetch o_proj weights: land during phase B's DMA-quiet stretch
        wo_s = []
        for dq in range(4):
            w = pool.tile([128, 4, 512], bf16, tag=f"wo{dq}", name=f"wo{dq}")
            nc.gpsimd.dma_start(w[:], wot[dq])
            wo_s.append(w)
        psS = phaseB.enter_context(tc.tile_pool(name="psS", bufs=4, space="PSUM"))
        psO = phaseB.enter_context(tc.tile_pool(name="psO", bufs=2, space="PSUM"))
        psD = phaseB.enter_context(tc.tile_pool(name="psD", bufs=2, space="PSUM"))
        pt_pool = phaseB.enter_context(tc.tile_pool(name="pt", bufs=6))
        rc_pool = phaseB.enter_context(tc.tile_pool(name="rc", bufs=2))

        for h in range(HD):
            for c in range(SC):
                nblk = 4 * c + 4
                po = psO.tile([128, 512], f32, tag="po", name=f"po{h}_{c}")
                pd = psD.tile([128, 512], f32, tag="pd", name=f"pd{h}_{c}")
                pts = [None] * nblk
                # software pipeline: QK/exp two blocks ahead of AV/denom so
                # the DVE mask-add + ACT exp latency never stalls the PE
                for skb in range(nblk + 2):
                    if skb < nblk:
                        ps = psS.tile([128, 512], f32, tag="ps",
                                      name=f"ps{h}_{c}_{skb}")
                        nc.tensor.matmul(
                            ps[:],
                            kt16[skb // 4][:, 128 * (skb % 4):128 * (skb % 4 + 1)],
                            qt[h][c][:], start=True, stop=True)
                        if skb >= 4 * c:
                            nc.vector.tensor_add(ps[:], ps[:],
                                                 mask_s[:, skb - 4 * c, :])
                        pt = pt_pool.tile([128, 512], bf16, tag="pt",
                                          name=f"pt{h}_{c}_{skb}")
                        nc.scalar.activation(pt[:], ps[:], AF.Exp,
                                             scale=float(SCALE))
                        pts[skb] = pt
                    if skb >= 2:
                        j = skb - 2
                        pr = pts[j][:]
                        nc.tensor.matmul(po[:], v_s[j][:], pr,
                                         start=(j == 0), stop=(j == nblk - 1))
                        nc.tensor.matmul(pd[:], ones_s[:], pr,
                                         start=(j == 0), stop=(j == nblk - 1))
                rc = rc_pool.tile([128, 512], f32, tag="rc", name=f"rc{h}_{c}")
                nc.vector.reciprocal(rc[:], pd[:])
                nc.vector.tensor_mul(ot[h][c][:], po[:], rc[:])
        phaseB.close()

        # ---------------- Phase C: o_proj (partial over this head group) ----
        psC = ctx.enter_context(tc.tile_pool(name="psC", bufs=2, space="PSUM"))
        ev_pool = ctx.enter_context(tc.tile_pool(name="ev", bufs=4))
        for sb in range(SB):
            pos = [psC.tile([128, 512], f32, tag=f"pc{e}", name=f"pc{e}_{sb}")
                   for e in range(4)]
            for dq in range(HD):
                lh = ot[dq][sb // 4][:, 128 * (sb % 4):128 * (sb % 4 + 1)]
                for ec in range(4):
                    nc.tensor.matmul(pos[ec][:], lh, wo_s[dq][:, ec, :],
                                     start=(dq == 0), stop=(dq == HD - 1))
            for ec in range(4):
                ev = ev_pool.tile([128, 512], f32, tag="ev",
                                  name=f"ev{sb}_{ec}")
                if ec % 2 == 0:
                    nc.scalar.copy(ev[:], pos[ec][:])
                else:
                    nc.vector.tensor_copy(ev[:], pos[ec][:])
                eng = nc.sync if ec % 2 == 0 else nc.gpsimd
                eng.dma_start(
                    outp[128 * sb:128 * (sb + 1), 512 * ec:512 * (ec + 1)],
                    ev[:])

    nc.compile()
    return nc


def _prep_core(b, g, x, rope_cos, rope_sin, Wq, bq, Wk, bk, Wv, bv, Wo,
               masks, ones, ident):
    f = np.float32
    b16 = ml_dtypes.bfloat16
    c = np.ascontiguousarray
    sl = slice(D * HD * g, D * HD * (g + 1))
    kv = slice(D * g, D * (g + 1))
    sin = c(rope_sin[b, :, 0, :].T.astype(f))   # [D, S]
    sin[0:D // 2, :] *= -1.0
    wqT = Wq[sl, :].T.astype(b16)                      # [H, 512]
    wkT = Wk[kv, :].T.astype(b16)                      # [H, 128]
    wvT = Wv[kv, :].T.astype(b16)
    woT = Wo[:, sl].T.astype(b16)                      # [512, 2048]
    return {
        "xT": c(x[b].T.astype(b16)),
        "wqt": c(wqT.reshape(8, 2, D, HD * D).transpose(0, 2, 1, 3)),
        "wkt": c(wkT.reshape(4, 4, D, D).transpose(0, 2, 1, 3)),
        "wvt": c(wvT.reshape(4, 4, D, D).transpose(0, 2, 1, 3)),
        "wot": c(woT.reshape(4, D, 4, 512).transpose(0, 1, 2, 3)),
        "bq": c(bq[sl].reshape(HD, D).T.astype(f)),
        "bk": c(bk[kv].reshape(D, 1).astype(f)),
        "bv": c(bv[kv].reshape(D, 1).astype(f)),
        "cosT": c(rope_cos[b, :, 0, :].T.astype(f)),
        "sinT": sin,
        "maskT": masks,
        "ones": ones,
        "ident": ident,
    }


def kernel(x, rope_cos, rope_sin, Wq, bq, Wk, bk, Wv, bv, Wo):
    global LAST_EXEC_NS
    from concourse.bass_utils import run_bass_kernel_spmd

    if "nc" not in _CACHE:
        _CACHE["nc"] = _build_nc()
    nc = _CACHE["nc"]

    # causal mask tiles for the 4 diagonal sub-blocks of each 512-chunk
    p = np.arange(D)[:, None]
    fidx = np.arange(512)[None, :]
    masks = np.stack(
        [np.where(128 * j + p <= fidx, 0.0, NEG).astype(np.float32)
         for j in range(4)])
    ones = np.ones((D, D), ml_dtypes.bfloat16)
    ident = np.eye(D, dtype=np.float32)

    in_maps = [
        _prep_core(core // NKV, core % NKV, x, rope_cos, rope_sin,
                   Wq, bq, Wk, bk, Wv, bv, Wo, masks, ones, ident)
        for core in range(N_CORES)
    ]

    trace = bool(int(os.environ.get("BASS_GQA_TRACE", "0")))
    res = run_bass_kernel_spmd(nc, in_maps, core_ids=list(range(N_CORES)),
                               trace=trace)
    LAST_EXEC_NS = res.exec_time_ns

    out = np.zeros((B, S, H), np.float32)
    new_k = np.empty((B, NKV, S, D), np.float32)
    new_v = np.empty((B, NKV, S, D), np.float32)
    for core in range(N_CORES):
        b, g = core // NKV, core % NKV
        r = res.results[core]
        out[b] += np.asarray(r["outp"], np.float32)
        new_k[b, g] = np.asarray(r["kT_out"], np.float32).T
        new_v[b, g] = np.asarray(r["vT_out"], np.float32).T
    return out, new_k, new_v


# revision 22
# speedup vs baseline: 1.0360x; 1.0360x over previous
"""GQA self-attention (B=2, S=2048, H=2048, NQ=16, NKV=4, D=128) on 8 TRN2
NeuronCores.

Sharding: core = (batch, kv-group): 2 batches x 4 kv heads. Each core computes
its batch's 4 q heads + 1 kv head end-to-end (q/k/v proj + rope + causal
attention + partial o_proj against the matching 512-column slice of Wo).
Host sums the 4 partial o_proj outputs per batch (the "all-reduce") and
reassembles new_k / new_v.

Device-side layout: everything feature-on-partitions. Host pre-transposes
x -> xT [H, S] and weights -> W^T so every matmul contraction dim is the
partition dim. Matmul operands are bf16 (fp32 accumulation in PSUM): enables
fast-weight-load and halves DMA; new_k/new_v are produced from the fp32
pre-cast intermediates so the cache outputs keep full precision. Attention
scores are computed transposed ([sk, sq]) so the exp'd probabilities feed the
A@V matmul directly as the moving operand; the softmax denominator comes from
a ones-stationary matmul accumulated in PSUM (broadcast across partitions for
a full-width reciprocal+scale). Softmax skips the max-subtraction
(scores*scale are O(10); exp cannot overflow and softmax is shift-invariant).
V is projected transposed like q/k and flipped back to [s, d] via PE
transposes (full-width N=512 matmuls instead of N=128 ones).
"""

import os
import sys

for _p in ("/opt/trn_rl_repo",):
    if _p not in sys.path and os.path.isdir(_p):
        sys.path.insert(0, _p)

import numpy as np
import ml_dtypes

B, S, H = 2, 2048, 2048
NQ, NKV, D = 16, 4, 128
N_CORES = 8
HD = 4          # q heads per core
KC = H // 128   # 16 hidden chunks
SC = S // 512   # 4 sequence 512-chunks
SB = S // 128   # 16 sequence 128-blocks
SCALE = 1.0 / np.sqrt(D).astype(np.float32)
NEG = -1.0e30

_CACHE = {}
LAST_EXEC_NS = None


def _build_nc():
    from contextlib import ExitStack

    from concourse import bacc, tile
    import concourse.mybir as mybir

    f32 = mybir.dt.float32
    bf16 = mybir.dt.bfloat16
    AF = mybir.ActivationFunctionType

    nc = bacc.Bacc("TRN2", target_bir_lowering=False, debug=False,
                   num_devices=N_CORES)

    def din(name, shape, dt=f32):
        return nc.dram_tensor(name, list(shape), dt, kind="ExternalInput").ap()

    def dout(name, shape, dt=f32):
        return nc.dram_tensor(name, list(shape), dt, kind="ExternalOutput").ap()

    xT = din("xT", (H, S), bf16)          # x[b].T
    # weights pre-tiled on host so each DMA moves >=1KB-contiguous lines
    wqt = din("wqt", (8, D, 2, HD * D), bf16)   # pairs of h-chunks of Wq.T
    wkt = din("wkt", (4, D, 4, D), bf16)        # quads of h-chunks of Wk.T
    wvt = din("wvt", (4, D, 4, D), bf16)
    wot = din("wot", (4, D, 4, 512), bf16)      # [dq-chunk][d][ec][e']
    bq = din("bq", (D, HD))               # bq[g-slice] as [d, head]
    bk = din("bk", (D, 1))
    bv = din("bv", (D, 1))
    cosT = din("cosT", (D, S))            # rope_cos[b].T
    sinT = din("sinT", (D, S))            # rope_sin[b].T, rows 0:64 negated
    maskT = din("maskT", (4, D, 512))     # causal mask tiles for diag blocks
    ones = din("ones", (D, D), bf16)
    ident = din("ident", (D, D))          # identity for PE transposes

    outp = dout("outp", (S, H))           # partial out[b] (this group's slice)
    kT_out = dout("kT_out", (D, S))       # rope'd k, transposed, fp32
    vT_out = dout("vT_out", (D, S))       # v, transposed, fp32

    with tile.TileContext(nc) as tc, ExitStack() as ctx:
        pool = ctx.enter_context(tc.tile_pool(name="persist", bufs=1))

        wq_s = []
        for kp in range(8):
            w = pool.tile([128, 2, HD * D], bf16, tag=f"wq{kp}", name=f"wq{kp}")
            nc.gpsimd.dma_start(w[:], wqt[kp])
            wq_s.append(w)
        wk_s = []
        wv_s = []
        for kp in range(4):
            w = pool.tile([128, 4, D], bf16, tag=f"wk{kp}", name=f"wk{kp}")
            nc.scalar.dma_start(w[:], wkt[kp])
            wk_s.append(w)
            w = pool.tile([128, 4, D], bf16, tag=f"wv{kp}", name=f"wv{kp}")
            nc.scalar.dma_start(w[:], wvt[kp])
            wv_s.append(w)

        def wq_sl(kc, h):
            return wq_s[kc // 2][:, kc % 2, 128 * h:128 * (h + 1)]

        def wk_sl(kc):
            return wk_s[kc // 4][:, kc % 4, :]

        def wv_sl(kc):
            return wv_s[kc // 4][:, kc % 4, :]
        cos_s = pool.tile([128, S], f32)
        sin_s = pool.tile([128, S], f32)
        mask_s = pool.tile([128, 4, 512], f32)
        ones_s = pool.tile([128, D], bf16)
        ident_s = pool.tile([128, D], f32)
        bq_s = pool.tile([128, HD], f32)
        bk_s = pool.tile([128, 1], f32)
        bv_s = pool.tile([128, 1], f32)

        def load_consts():
            # issued from the scalar sequencer so they do
            # not delay the SP-issued xt loads that gate the first matmuls
            nc.scalar.dma_start(bq_s[:], bq[:, :])
            nc.scalar.dma_start(bk_s[:], bk[:, :])
            nc.scalar.dma_start(bv_s[:], bv[:, :])
            nc.scalar.dma_start(cos_s[:], cosT[:, :])
            nc.scalar.dma_start(sin_s[:], sinT[:, :])
            nc.scalar.dma_start(ident_s[:], ident[:, :])
            nc.scalar.dma_start(mask_s[:], maskT.rearrange("j p f -> p j f"))
            nc.scalar.dma_start(ones_s[:], ones[:, :])

        # chunked activations (per-512-chunk tiles -> fine-grained deps)
        qt = [[pool.tile([128, 512], bf16, tag=f"qt{h}_{c}", name=f"qt{h}_{c}")
               for c in range(SC)] for h in range(HD)]
        ktf = [pool.tile([128, 512], f32, tag=f"ktf{c}", name=f"ktf{c}")
               for c in range(SC)]
        kt16 = [pool.tile([128, 512], bf16, tag=f"kt16_{c}", name=f"kt16_{c}")
                for c in range(SC)]
        vtf = [pool.tile([128, 512], f32, tag=f"vtf{c}", name=f"vtf{c}")
               for c in range(SC)]
        v_s = [pool.tile([128, D], bf16, tag=f"v{sb}", name=f"v{sb}")
               for sb in range(SB)]
        ot = [[pool.tile([128, 512], bf16, tag=f"ot{h}_{c}", name=f"ot{h}_{c}")
               for c in range(SC)] for h in range(HD)]

        tmp_pool = ctx.enter_context(tc.tile_pool(name="tmp", bufs=3))

        def rope_evac(dst, psum, bias_ap, cs, nm):
            # dst = (psum + bias) * cos + swap_halves(psum + bias) * sin_signed
            qb = tmp_pool.tile([128, 512], f32, tag="ropeqb", name=f"qb_{nm}")
            nc.scalar.activation(qb[:], psum, AF.Identity, bias=bias_ap)
            qsw = tmp_pool.tile([128, 512], f32, tag="ropesw", name=f"sw_{nm}")
            nc.gpsimd.dma_start(qsw[0:64, :], qb[64:128, :])
            nc.gpsimd.dma_start(qsw[64:128, :], qb[0:64, :])
            t = tmp_pool.tile([128, 512], f32, tag="ropet", name=f"t_{nm}")
            nc.vector.tensor_mul(t[:], qsw[:], sin_s[:, cs])
            t2 = tmp_pool.tile([128, 512], f32, tag="ropet2", name=f"t2_{nm}")
            nc.vector.tensor_mul(t2[:], qb[:], cos_s[:, cs])
            nc.vector.tensor_add(dst, t2[:], t[:])

        # ---------------- Phase A: q/k/v projections + rope ----------------
        phaseA = ExitStack()
        xt_pool = phaseA.enter_context(tc.tile_pool(name="xt", bufs=24))
        psA = phaseA.enter_context(tc.tile_pool(name="psA", bufs=1, space="PSUM"))
        psT = phaseA.enter_context(tc.tile_pool(name="psT", bufs=2, space="PSUM"))
        for cpair in range(SC // 2):
          xts = []
          for kc in range(KC):
              xt = xt_pool.tile([128, 1024], bf16, tag="xt",
                                name=f"xt{kc}_{cpair}")
              nc.sync.dma_start(
                  xt[:], xT[128 * kc:128 * (kc + 1),
                            1024 * cpair:1024 * (cpair + 1)])
              xts.append(xt)
          if cpair == 0:
              load_consts()
          for ci in range(2):
            c = 2 * cpair + ci
            cs = slice(512 * c, 512 * (c + 1))
            pq = [psA.tile([128, 512], f32, tag=f"pq{h}", name=f"pq{h}_{c}")
                  for h in range(HD)]
            pk = psA.tile([128, 512], f32, tag="pk", name=f"pk_{c}")
            pv = psA.tile([128, 512], f32, tag="pv", name=f"pv_{c}")
            for kc in range(KC):
                st, sp = kc == 0, kc == KC - 1
                xr = xts[kc][:, 512 * ci:512 * (ci + 1)]
                for h in range(HD):
                    nc.tensor.matmul(pq[h][:], wq_sl(kc, h), xr,
                                     start=st, stop=sp)
                nc.tensor.matmul(pk[:], wk_sl(kc), xr, start=st, stop=sp)
                nc.tensor.matmul(pv[:], wv_sl(kc), xr, start=st, stop=sp)
            for h in range(HD):
                rope_evac(qt[h][c][:], pq[h][:], bq_s[:, h:h + 1], cs,
                          f"q{h}_{c}")
            rope_evac(ktf[c][:], pk[:], bk_s[:, :], cs, f"k_{c}")
            nc.vector.tensor_copy(kt16[c][:], ktf[c][:])
            nc.gpsimd.dma_start(kT_out[:, cs], ktf[c][:])
            # v: add bias on evac, write fp32 cache copy, then PE-transpose
            # each 128-block into [s, d] layout for the A@V stationary.
            nc.scalar.activation(vtf[c][:], pv[:], AF.Identity, bias=bv_s[:, :])
            nc.gpsimd.dma_start(vT_out[:, cs], vtf[c][:])
            for j in range(4):
                ptr = psT.tile([128, D], f32, tag="ptr", name=f"ptr{c}_{j}")
                nc.tensor.transpose(ptr[:], vtf[c][:, 128 * j:128 * (j + 1)],
                                    ident_s[:])
                nc.scalar.copy(v_s[4 * c + j][:].bitcast(bf16), ptr[:])
        phaseA.close()

        # ---------------- Phase B: causal attention per head ----------------
        phaseB = ExitStack()
        # prefetch o_proj weights: land during phase B's DMA-quiet stretch
        wo_s = []
        for dq in range(4):
            w = pool.tile([128, 4, 512], bf16, tag=f"wo{dq}", name=f"wo{dq}")
            nc.gpsimd.dma_start(w[:], wot[dq])
            wo_s.append(w)
        psS = phaseB.enter_context(tc.tile_pool(name="psS", bufs=4, space="PSUM"))
        psO = phaseB.enter_context(tc.tile_pool(name="psO", bufs=2, space="PSUM"))
        psD = phaseB.enter_context(tc.tile_pool(name="psD", bufs=2, space="PSUM"))
        pt_pool = phaseB.enter_context(tc.tile_pool(name="pt", bufs=6))
        rc_pool = phaseB.enter_context(tc.tile_pool(name="rc", bufs=2))

        for h in range(HD):
            for c in range(SC):
                nblk = 4 * c + 4
                po = psO.tile([128, 512], f32, tag="po", name=f"po{h}_{c}")
                pd = psD.tile([128, 512], f32, tag="pd", name=f"pd{h}_{c}")
                pts = [None] * nblk
                # software pipeline: QK/exp two blocks ahead of AV/denom so
                # the DVE mask-add + ACT exp latency never stalls the PE
                for skb in range(nblk + 2):
                    if skb < nblk:
                        ps = psS.tile([128, 512], f32, tag="ps",
                                      name=f"ps{h}_{c}_{skb}")
                        nc.tensor.matmul(
                            ps[:],
                            kt16[skb // 4][:, 128 * (skb % 4):128 * (skb % 4 + 1)],
                            qt[h][c][:], start=True, stop=True)
                        if skb >= 4 * c:
                            nc.vector.tensor_add(ps[:], ps[:],
                                                 mask_s[:, skb - 4 * c, :])
                        pt = pt_pool.tile([128, 512], bf16, tag="pt",
                                          name=f"pt{h}_{c}_{skb}")
                        nc.scalar.activation(pt[:], ps[:], AF.Exp,
                                             scale=float(SCALE))
                        pts[skb] = pt
                    if skb >= 2:
                        j = skb - 2
                        pr = pts[j][:]
                        nc.tensor.matmul(po[:], v_s[j][:], pr,
                                         start=(j == 0), stop=(j == nblk - 1))
                        nc.tensor.matmul(pd[:], ones_s[:], pr,
                                         start=(j == 0), stop=(j == nblk - 1))
                rc = rc_pool.tile([128, 512], f32, tag="rc", name=f"rc{h}_{c}")
                nc.vector.reciprocal(rc[:], pd[:])
                nc.vector.tensor_mul(ot[h][c][:], po[:], rc[:])
        phaseB.close()

        # ---------------- Phase C: o_proj (partial over this head group) ----
        psC = ctx.enter_context(tc.tile_pool(name="psC", bufs=2, space="PSUM"))
        ev_pool = ctx.enter_context(tc.tile_pool(name="ev", bufs=4))
        for sb in range(SB):
            pos = [psC.tile([128, 512], f32, tag=f"pc{e}", name=f"pc{e}_{sb}")
                   for e in range(4)]
            for dq in range(HD):
                lh = ot[dq][sb // 4][:, 128 * (sb % 4):128 * (sb % 4 + 1)]
                for ec in range(4):
                    nc.tensor.matmul(pos[ec][:], lh, wo_s[dq][:, ec, :],
                                     start=(dq == 0), stop=(dq == HD - 1))
            for ec in range(4):
                ev = ev_pool.tile([128, 512], f32, tag="ev",
                                  name=f"ev{sb}_{ec}")
                if ec % 2 == 0:
                    nc.scalar.copy(ev[:], pos[ec][:])
                else:
                    nc.vector.tensor_copy(ev[:], pos[ec][:])
                eng = nc.sync if ec % 2 == 0 else nc.gpsimd
                eng.dma_start(
                    outp[128 * sb:128 * (sb + 1), 512 * ec:512 * (ec + 1)],
                    ev[:])

    nc.compile()
    return nc


def _prep_core(b, g, x, rope_cos, rope_sin, Wq, bq, Wk, bk, Wv, bv, Wo,
               masks, ones, ident):
    f = np.float32
    b16 = ml_dtypes.bfloat16
    c = np.ascontiguousarray
    sl = slice(D * HD * g, D * HD * (g + 1))
    kv = slice(D * g, D * (g + 1))
    sin = c(rope_sin[b, :, 0, :].T.astype(f))   # [D, S]
    sin[0:D // 2, :] *= -1.0
    wqT = Wq[sl, :].T.astype(b16)                      # [H, 512]
    wkT = Wk[kv, :].T.astype(b16)                      # [H, 128]
    wvT = Wv[kv, :].T.astype(b16)
    woT = Wo[:, sl].T.astype(b16)                      # [512, 2048]
    return {
        "xT": c(x[b].T.astype(b16)),
        "wqt": c(wqT.reshape(8, 2, D, HD * D).transpose(0, 2, 1, 3)),
        "wkt": c(wkT.reshape(4, 4, D, D).transpose(0, 2, 1, 3)),
        "wvt": c(wvT.reshape(4, 4, D, D).transpose(0, 2, 1, 3)),
        "wot": c(woT.reshape(4, D, 4, 512).transpose(0, 1, 2, 3)),
        "bq": c(bq[sl].reshape(HD, D).T.astype(f)),
        "bk": c(bk[kv].reshape(D, 1).astype(f)),
        "bv": c(bv[kv].reshape(D, 1).astype(f)),
        "cosT": c(rope_cos[b, :, 0, :].T.astype(f)),
        "sinT": sin,
        "maskT": masks,
        "ones": ones,
        "ident": ident,
    }


def kernel(x, rope_cos, rope_sin, Wq, bq, Wk, bk, Wv, bv, Wo):
    global LAST_EXEC_NS
    from concourse.bass_utils import run_bass_kernel_spmd

    if "nc" not in _CACHE:
        _CACHE["nc"] = _build_nc()
    nc = _CACHE["nc"]

    # causal mask tiles for the 4 diagonal sub-blocks of each 512-chunk
    p = np.arange(D)[:, None]
    fidx = np.arange(512)[None, :]
    masks = np.stack(
        [np.where(128 * j + p <= fidx, 0.0, NEG).astype(np.float32)
         for j in range(4)])
    ones = np.ones((D, D), ml_dtypes.bfloat16)
    ident = np.eye(D, dtype=np.float32)

    in_maps = [
        _prep_core(core // NKV, core % NKV, x, rope_cos, rope_sin,
                   Wq, bq, Wk, bk, Wv, bv, Wo, masks, ones, ident)
        for core in range(N_CORES)
    ]

    trace = bool(int(os.environ.get("BASS_GQA_TRACE", "0")))
    res = run_bass_kernel_spmd(nc, in_maps, core_ids=list(range(N_CORES)),
                               trace=trace)
    LAST_EXEC_NS = res.exec_time_ns

    out = np.zeros((B, S, H), np.float32)
    new_k = np.empty((B, NKV, S, D), np.float32)
    new_v = np.empty((B, NKV, S, D), np.float32)
    for core in range(N_CORES):
        b, g = core // NKV, core % NKV
        r = res.results[core]
        out[b] += np.asarray(r["outp"], np.float32)
        new_k[b, g] = np.asarray(r["kT_out"], np.float32).T
        new_v[b, g] = np.asarray(r["vT_out"], np.float32).T
    return out, new_k, new_v
